# revision 7
# baseline (speedup 1.0000x reference)
"""Batched dense attention (B=16, S=2048, E=128, fp32) on 8 TRN2 NeuronCores.

Strategy (data-parallel over batch, 2 batch elements per core):
  - Load Q, K naturally ([s,e] -> SBUF [128, 2048]), PE-transpose to
    QT/KT [e=128, s=2048] (fp32).
  - scores^T tiles [k=128, q=512] = KT_tile.T @ QT_chunk via float32r
    matmuls (full rate at N=512).
  - exp on ScalarE reading PSUM, scale=1/sqrt(E) folded in, bf16 out.
    No max subtraction (scores ~ N(0,1); exp is safely bounded in fp32).
  - P@V via bf16 matmuls, lhsT = exp(scores^T) subtile [k=128, q=128],
    rhs = V' = [V | ones] [k=128, 129]; column 128 accumulates the
    softmax denominator for free.  Accumulate over k in PSUM.
  - Normalize per-partition with DVE reciprocal + tensor_scalar_mul.
"""

import numpy as np
from contextlib import ExitStack

import concourse.bass as bass
import concourse.tile as tile
from concourse import bacc, bass_utils, mybir
from concourse.masks import make_identity

B, S, E = 16, 2048, 128
N_CORES = 8
B_LOC = B // N_CORES          # batch elems per core
P = 128                       # partitions
NT = S // P                   # 16 s-tiles per batch elem
QCHUNK = 512
NQC = S // QCHUNK             # 4 q-chunks
SCALE = float(E) ** -0.5

f32 = mybir.dt.float32
f32r = mybir.dt.float32r
bf16 = mybir.dt.bfloat16
AF = mybir.ActivationFunctionType


def emit_attention(ctx: ExitStack, tc: tile.TileContext, out_ap, q_ap, k_ap, v_ap):
    nc = tc.nc

    const_pool = ctx.enter_context(tc.tile_pool(name="const", bufs=1))
    ident = const_pool.tile([P, P], f32)
    make_identity(nc, ident)

    stage_pool = ctx.enter_context(tc.tile_pool(name="stage", bufs=4))
    qt_pool = ctx.enter_context(tc.tile_pool(name="qt", bufs=2))
    kt_pool = ctx.enter_context(tc.tile_pool(name="kt", bufs=2))
    vv_pool = ctx.enter_context(tc.tile_pool(name="vv", bufs=2))
    ex_pool = ctx.enter_context(tc.tile_pool(name="ex", bufs=3))
    osb_pool = ctx.enter_context(tc.tile_pool(name="osb", bufs=2))
    rcp_pool = ctx.enter_context(tc.tile_pool(name="rcp", bufs=8))
    spsum_pool = ctx.enter_context(tc.tile_pool(name="spsum", bufs=2, space="PSUM"))
    opsum_pool = ctx.enter_context(tc.tile_pool(name="opsum", bufs=4, space="PSUM"))

    for b in range(B_LOC):
        # ---- load inputs ----
        q_nat = stage_pool.tile([P, NT, E], f32, tag="stage", name=f"q_nat{b}")
        nc.sync.dma_start(q_nat[:], q_ap[b].rearrange("(n p) e -> p n e", p=P))
        k_nat = stage_pool.tile([P, NT, E], f32, tag="stage", name=f"k_nat{b}")
        nc.sync.dma_start(k_nat[:], k_ap[b].rearrange("(n p) e -> p n e", p=P))
        # V' = [V | ones], bf16, one [128, 129] tile per k-tile
        vv = vv_pool.tile([P, NT, E + 1], bf16, name=f"vv{b}")
        nc.gpsimd.dma_start(
            vv[:, :, 0:E], v_ap[b].rearrange("(n p) e -> p n e", p=P)
        )
        nc.vector.memset(vv[:, :, E : E + 1], 1.0)

        # ---- transpose Q, K to [e, s] ----
        qt = qt_pool.tile([P, S], f32r, name=f"qt{b}")
        kt = kt_pool.tile([P, S], f32r, name=f"kt{b}")
        for src, dst in ((q_nat, qt), (k_nat, kt)):
            for half in range(2):
                tp = spsum_pool.tile([P, 2 * QCHUNK], f32, tag="spsum", name="tp")
                for i in range(8):
                    n = half * 8 + i
                    nc.tensor.transpose(
                        tp[:, i * P : (i + 1) * P],
                        src[:, n, :],
                        ident[:],
                    )
                nc.vector.tensor_copy(
                    dst[:, half * 2 * QCHUNK : (half + 1) * 2 * QCHUNK], tp[:]
                )

        osb = osb_pool.tile([P, NT, E], f32, name=f"osb{b}")

        # ---- main attention loop ----
        for qc in range(NQC):
            accs = [
                opsum_pool.tile([P, E + 1], f32, tag="acc", name=f"acc{qs}")
                for qs in range(4)
            ]
            for kp in range(NT // 2):
                sc = spsum_pool.tile([P, 2 * QCHUNK], f32, tag="spsum", name="sc")
                for j in range(2):
                    ktile = kp * 2 + j
                    nc.tensor.matmul(
                        sc[:, j * QCHUNK : (j + 1) * QCHUNK],
                        kt[:, ktile * P : (ktile + 1) * P],
                        qt[:, qc * QCHUNK : (qc + 1) * QCHUNK],
                        start=True,
                        stop=True,
                    )
                ex = ex_pool.tile([P, 2 * QCHUNK], bf16, name="ex")
                nc.scalar.activation(ex[:], sc[:], AF.Exp, scale=SCALE)
                for j in range(2):
                    ktile = kp * 2 + j
                    for qs in range(4):
                        nc.tensor.matmul(
                            accs[qs][:],
                            ex[:, j * QCHUNK + qs * P : j * QCHUNK + (qs + 1) * P],
                            vv[:, ktile, :],
                            start=(ktile == 0),
                            stop=(ktile == NT - 1),
                        )
            for qs in range(4):
                rcp = rcp_pool.tile([P, 1], f32, name="rcp")
                nc.vector.reciprocal(rcp[:], accs[qs][:, E : E + 1])
                nc.vector.tensor_scalar_mul(
                    osb[:, qc * 4 + qs, :],
                    accs[qs][:, 0:E],
                    rcp[:],
                )

        nc.sync.dma_start(out_ap[b].rearrange("(n p) e -> p n e", p=P), osb[:])


_CACHE: dict = {}


def build():
    if "nc" in _CACHE:
        return _CACHE["nc"]
    nc = bacc.Bacc(
        "TRN2",
        target_bir_lowering=False,
        debug=False,
        enable_asserts=False,
        num_devices=N_CORES,
    )
    q = nc.dram_tensor("q", [B_LOC, S, E], f32, kind="ExternalInput").ap()
    k = nc.dram_tensor("k", [B_LOC, S, E], f32, kind="ExternalInput").ap()
    v = nc.dram_tensor("v", [B_LOC, S, E], f32, kind="ExternalInput").ap()
    o = nc.dram_tensor("out", [B_LOC, S, E], f32, kind="ExternalOutput").ap()
    with tile.TileContext(nc) as tc, ExitStack() as ctx:
        emit_attention(ctx, tc, o, q, k, v)
    nc.compile()
    _CACHE["nc"] = nc
    return nc


def run(query, key, value, trace=False, trace_kwargs=None):
    nc = build()
    query = np.ascontiguousarray(query, dtype=np.float32)
    key = np.ascontiguousarray(key, dtype=np.float32)
    value = np.ascontiguousarray(value, dtype=np.float32)
    in_maps = [
        {
            "q": query[c * B_LOC : (c + 1) * B_LOC],
            "k": key[c * B_LOC : (c + 1) * B_LOC],
            "v": value[c * B_LOC : (c + 1) * B_LOC],
        }
        for c in range(N_CORES)
    ]
    res = bass_utils.run_bass_kernel_spmd(
        nc,
        in_maps,
        core_ids=list(range(N_CORES)),
        trace=trace,
        **(trace_kwargs or {}),
    )
    out = np.concatenate([res.results[c]["out"] for c in range(N_CORES)], axis=0)
    return out, res


def kernel(query, key, value):
    out, _ = run(query, key, value, trace=False)
    return out


# revision 11
# speedup vs baseline: 1.0200x; 1.0200x over previous
"""Batched dense attention (B=16, S=2048, E=128, fp32) on 8 TRN2 NeuronCores.

Strategy (data-parallel over batch, 2 batch elements per core):
  - Load Q, K naturally ([s,e] -> SBUF [128, 2048]), PE-transpose to
    QT/KT [e=128, s=2048] (fp32).
  - scores^T tiles [k=128, q=512] = KT_tile.T @ QT_chunk via float32r
    matmuls (full rate at N=512).
  - exp on ScalarE reading PSUM, scale=1/sqrt(E) folded in, bf16 out.
    No max subtraction (scores ~ N(0,1); exp is safely bounded in fp32).
  - P@V via bf16 matmuls, lhsT = exp(scores^T) subtile [k=128, q=128],
    rhs = V' = [V | ones] [k=128, 129]; column 128 accumulates the
    softmax denominator for free.  Accumulate over k in PSUM.
  - Normalize per-partition with DVE reciprocal + tensor_scalar_mul.
"""

import numpy as np
from contextlib import ExitStack

import concourse.bass as bass
import concourse.tile as tile
from concourse import bacc, bass_utils, mybir
from concourse.masks import make_identity

B, S, E = 16, 2048, 128
N_CORES = 8
B_LOC = B // N_CORES          # batch elems per core
P = 128                       # partitions
NT = S // P                   # 16 s-tiles per batch elem
QCHUNK = 512
NQC = S // QCHUNK             # 4 q-chunks
SCALE = float(E) ** -0.5

f32 = mybir.dt.float32
f32r = mybir.dt.float32r
bf16 = mybir.dt.bfloat16
AF = mybir.ActivationFunctionType


def emit_attention(ctx: ExitStack, tc: tile.TileContext, out_ap, q_ap, k_ap, v_ap):
    nc = tc.nc

    const_pool = ctx.enter_context(tc.tile_pool(name="const", bufs=1))
    ident = const_pool.tile([P, P], f32)
    make_identity(nc, ident)

    stage_pool = ctx.enter_context(tc.tile_pool(name="stage", bufs=4))
    qt_pool = ctx.enter_context(tc.tile_pool(name="qt", bufs=2))
    kt_pool = ctx.enter_context(tc.tile_pool(name="kt", bufs=2))
    vv_pool = ctx.enter_context(tc.tile_pool(name="vv", bufs=2))
    ex_pool = ctx.enter_context(tc.tile_pool(name="ex", bufs=4))
    osb_pool = ctx.enter_context(tc.tile_pool(name="osb", bufs=2))
    rcp_pool = ctx.enter_context(tc.tile_pool(name="rcp", bufs=8))
    spsum_pool = ctx.enter_context(tc.tile_pool(name="spsum", bufs=2, space="PSUM"))
    opsum_pool = ctx.enter_context(tc.tile_pool(name="opsum", bufs=4, space="PSUM"))

    # Software pipeline: the P@V matmuls (and chunk epilogue) for iteration
    # (qc, kp) are emitted one kp step later, so the PE always has the next
    # scores matmuls in front of a dependency-stalled mm2.
    carry = [None]

    EP = E + 1  # 129

    def emit_mm2(c):
        ex, vv, accs, kp = c["ex"], c["vv"], c["accs"], c["kp"]
        for j in range(2):
            ktile = kp * 2 + j
            for qs in range(4):
                # two q-subtiles share one PSUM bank (one accumulation group)
                acc = accs[qs // 2][:, (qs % 2) * EP : (qs % 2) * EP + EP]
                nc.tensor.matmul(
                    acc,
                    ex[:, j * QCHUNK + qs * P : j * QCHUNK + (qs + 1) * P],
                    vv[:, ktile, :],
                    start=(ktile == 0 and qs % 2 == 0),
                    stop=(ktile == NT - 1 and qs % 2 == 1),
                )
        if c["last"]:
            # chunk epilogue: normalize + store this q-chunk
            accs, osb, qc, out_dr = c["accs"], c["osb"], c["qc"], c["out_dr"]
            for qs in range(4):
                rcp = rcp_pool.tile([P, 1], f32, name="rcp")
                nc.vector.reciprocal(
                    rcp[:], accs[qs // 2][:, (qs % 2) * EP + E : (qs % 2) * EP + E + 1]
                )
                nc.vector.tensor_scalar_mul(
                    osb[:, qc * 4 + qs, :],
                    accs[qs // 2][:, (qs % 2) * EP : (qs % 2) * EP + E],
                    rcp[:],
                )
            sl = slice(qc * 4, (qc + 1) * 4)
            nc.sync.dma_start(out_dr[:, sl, :], osb[:, sl, :])

    def flush():
        if carry[0] is not None:
            emit_mm2(carry[0])
            carry[0] = None

    NLC = 4  # input-DMA / transpose chunks (4 s-tiles each)
    TPC = NT // NLC

    for b in range(B_LOC):
        # ---- load inputs (chunked, K first: scores need all of K early) ----
        q_nat = stage_pool.tile([P, NT, E], f32, tag="stage", name=f"q_nat{b}")
        k_nat = stage_pool.tile([P, NT, E], f32, tag="stage", name=f"k_nat{b}")
        qdr = q_ap[b].rearrange("(n p) e -> p n e", p=P)
        kdr = k_ap[b].rearrange("(n p) e -> p n e", p=P)
        order = [("k", 0), ("q", 0), ("k", 1), ("k", 2), ("k", 3), ("q", 1), ("q", 2), ("q", 3)]
        for which, c in order:
            sl = slice(c * TPC, (c + 1) * TPC)
            if which == "q":
                nc.sync.dma_start(q_nat[:, sl, :], qdr[:, sl, :])
            else:
                nc.sync.dma_start(k_nat[:, sl, :], kdr[:, sl, :])
        # V' = [V | ones], bf16, one [128, 129] tile per k-tile
        vv = vv_pool.tile([P, NT, E + 1], bf16, name=f"vv{b}")
        nc.gpsimd.dma_start(
            vv[:, :, 0:E], v_ap[b].rearrange("(n p) e -> p n e", p=P)
        )
        nc.vector.memset(vv[:, :, E : E + 1], 1.0)

        # ---- transpose Q, K to [e, s] (per 4-tile chunk) ----
        qt = qt_pool.tile([P, S], f32r, name=f"qt{b}")
        kt = kt_pool.tile([P, S], f32r, name=f"kt{b}")
        tr_order = [("k", 0), ("q", 0), ("k", 1), ("k", 2), ("k", 3), ("q", 1), ("q", 2), ("q", 3)]
        for which, c in tr_order:
            src, dst = (q_nat, qt) if which == "q" else (k_nat, kt)
            tp = spsum_pool.tile([P, 2 * QCHUNK], f32, tag="spsum", name="tp")
            for i in range(TPC):
                n = c * TPC + i
                nc.tensor.transpose(tp[:, i * P : (i + 1) * P], src[:, n, :], ident[:])
            nc.vector.tensor_copy(
                dst[:, c * TPC * P : (c + 1) * TPC * P], tp[:, 0 : TPC * P]
            )

        osb = osb_pool.tile([P, NT, E], f32, name=f"osb{b}")
        out_dr = out_ap[b].rearrange("(n p) e -> p n e", p=P)

        # ---- main attention loop ----
        for qc in range(NQC):
            accs = [
                opsum_pool.tile([P, 2 * EP], f32, tag="acc", name=f"acc{qs}")
                for qs in range(2)
            ]
            for kp in range(NT // 2):
                sc = spsum_pool.tile([P, 2 * QCHUNK], f32, tag="spsum", name="sc")
                for j in range(2):
                    ktile = kp * 2 + j
                    nc.tensor.matmul(
                        sc[:, j * QCHUNK : (j + 1) * QCHUNK],
                        kt[:, ktile * P : (ktile + 1) * P],
                        qt[:, qc * QCHUNK : (qc + 1) * QCHUNK],
                        start=True,
                        stop=True,
                    )
                ex = ex_pool.tile([P, 2 * QCHUNK], bf16, name="ex")
                nc.scalar.activation(ex[:], sc[:], AF.Exp, scale=SCALE)
                flush()
                carry[0] = dict(
                    ex=ex,
                    vv=vv,
                    accs=accs,
                    kp=kp,
                    last=(kp == NT // 2 - 1),
                    osb=osb,
                    qc=qc,
                    out_dr=out_dr,
                )
    flush()


_CACHE: dict = {}


def build():
    if "nc" in _CACHE:
        return _CACHE["nc"]
    nc = bacc.Bacc(
        "TRN2",
        target_bir_lowering=False,
        debug=False,
        enable_asserts=False,
        num_devices=N_CORES,
    )
    q = nc.dram_tensor("q", [B_LOC, S, E], f32, kind="ExternalInput").ap()
    k = nc.dram_tensor("k", [B_LOC, S, E], f32, kind="ExternalInput").ap()
    v = nc.dram_tensor("v", [B_LOC, S, E], f32, kind="ExternalInput").ap()
    o = nc.dram_tensor("out", [B_LOC, S, E], f32, kind="ExternalOutput").ap()
    with tile.TileContext(nc) as tc, ExitStack() as ctx:
        emit_attention(ctx, tc, o, q, k, v)
    nc.compile()
    _CACHE["nc"] = nc
    return nc


def run(query, key, value, trace=False, trace_kwargs=None):
    nc = build()
    query = np.ascontiguousarray(query, dtype=np.float32)
    key = np.ascontiguousarray(key, dtype=np.float32)
    value = np.ascontiguousarray(value, dtype=np.float32)
    in_maps = [
        {
            "q": query[c * B_LOC : (c + 1) * B_LOC],
            "k": key[c * B_LOC : (c + 1) * B_LOC],
            "v": value[c * B_LOC : (c + 1) * B_LOC],
        }
        for c in range(N_CORES)
    ]
    res = bass_utils.run_bass_kernel_spmd(
        nc,
        in_maps,
        core_ids=list(range(N_CORES)),
        trace=trace,
        **(trace_kwargs or {}),
    )
    out = np.concatenate([res.results[c]["out"] for c in range(N_CORES)], axis=0)
    return out, res


def kernel(query, key, value):
    out, _ = run(query, key, value, trace=False)
    return out
